# revision 26
# baseline (speedup 1.0000x reference)
"""KAN layer kernel for TRN2, 8-core SPMD.

Math: out[b,o] = sum_{i,k} relu(x[b,i]*w1[o,i,k] + b1[o,i,k]) * w2[o,i,k] / 32 + b2[o]
With b1 == 0 (guaranteed by the generator) the relu factorizes:
    relu(x*w) = max(x,0)*max(w,0) + min(x,0)*min(w,0)
and with relu(-x) = relu(x) - x the layer collapses to two matmuls:
    T[o,i]  = sum_k w1*w2          S1[o,i] = sum_k relu(w1)*w2
    H = T - S1 ; G = S1 - H
    out = (relu(x) @ G^T + x @ H^T) / 32 + b2

Sharding: 4 batch groups x 2 dout groups (core = bi*2 + oj).
All tensors staged host-side in bf16 (halves DMA bytes, 1 cycle/row
matmuls, 2-4x DVE); accumulation in fp32 PSUM; bf16 output upcast on host.
relu(w1) runs at 4x DVE (tensor_scalar_max); one broadcast tensor_mul
computes w1*w2 and relu(w1)*w2 together at 2x.
"""

import numpy as np

B, DIN, DOUT, K = 2048, 256, 256, 4
N_CORES = 8
BG, OG = 4, 2                      # batch groups x dout groups
BS, OS = B // BG, DOUT // OG       # 512 batch rows, 128 dout cols per core
NT = DIN // 128                    # din tiles
HB = BS // 2                       # psum half of the batch shard
SCALE = 1.0 / np.sqrt(((DOUT + DIN) / 2) * K)   # 1/32
N_WARM = 40                        # PE warm-up matmuls ([128,128] each)

_CACHE = {}


def _build_nc():
    if "nc" in _CACHE:
        return _CACHE["nc"]
    import concourse.bacc as bacc
    import concourse.tile as tile
    from concourse import mybir

    f32 = mybir.dt.float32
    bf16 = mybir.dt.bfloat16
    AF = mybir.ActivationFunctionType
    OP = mybir.AluOpType

    nc = bacc.Bacc("TRN2", target_bir_lowering=False, debug=False,
                   num_devices=N_CORES, num_swdge_queues=2)
    # [i-part, i-tile, {w1,w2}, k, o]
    wt = nc.dram_tensor("wt", [128, NT, 2, K, OS], bf16, kind="ExternalInput")
    # [i-part, i-tile, b]
    xt = nc.dram_tensor("xt", [128, NT, BS], bf16, kind="ExternalInput")
    b2s = nc.dram_tensor("b2s", [OS, 1], f32, kind="ExternalInput")
    outt = nc.dram_tensor("outt", [OS, BS], bf16, kind="ExternalOutput")

    with tile.TileContext(nc) as tc:
        with (
            tc.tile_pool(name="io", bufs=1) as io,
            tc.tile_pool(name="work", bufs=1) as work,
            tc.tile_pool(name="pp", bufs=1, space="PSUM") as pp,
        ):
            # ---- DMA in: w0 first (feeds the serial DVE prep), x0 second
            # (feeds |x|/min(x,0) which fill DVE/ACT bubbles), then w1, x1.
            w_sb, x_sb = [], []
            for t in range(NT):
                # slots: 0=w1, 1=w2, 2=relu(w1) (filled on device)
                wi = io.tile([128, 3, K, OS], bf16, tag=f"w{t}")
                w_sb.append(wi)
                xi = io.tile([128, BS], bf16, tag=f"x{t}")
                x_sb.append(xi)
            # w0 split into w1/w2 slabs: relu(w1) starts one transfer earlier
            nc.sync.dma_start(out=w_sb[0][:, 0:1], in_=wt[:, 0, 0:1])
            nc.sync.dma_start(out=w_sb[0][:, 1:2], in_=wt[:, 0, 1:2])
            nc.sync.dma_start(out=x_sb[0], in_=xt[:, 0])
            nc.sync.dma_start(out=w_sb[1][:, 0:2], in_=wt[:, 1])
            nc.sync.dma_start(out=x_sb[1], in_=xt[:, 1])
            b2_sb = io.tile([OS, 1], f32)
            nc.sync.dma_start(out=b2_sb, in_=b2s[:, :])

            # ---- PE warm-up: keep the tensor engine busy from t~0 so the
            # real matmuls run at full clock (p-state ramp needs ~3us busy).
            zt = work.tile([128, 128], bf16, tag="zt")
            nc.gpsimd.memset(zt, 0.0)
            pz = pp.tile([128, 128], f32, tag="pz")
            for i in range(N_WARM):
                nc.tensor.matmul(pz, lhsT=zt, rhs=zt, start=True, stop=True)

            # ---- weight prep (DVE):
            #   w1p = relu(w1)                       (tensor_scalar_max, 4x)
            #   cat[:,0]=w1*w2, cat[:,1]=w1p*w2      (one broadcast mul, 2x)
            #   k-sum -> s[:,0]=T=sum_k w1*w2, s[:,1]=S1=sum_k relu(w1)*w2
            # out = |x| @ S1^T + min(x,0) @ T^T  (no G/H subtracts needed)
            s_t = []
            for t in range(NT):
                wi = w_sb[t]
                nc.vector.tensor_scalar_max(wi[:, 2], wi[:, 0], 0.0)
                cat = work.tile([128, 2, K, OS], bf16, tag=f"cat{t}")
                in0 = wi[:, 0::2]
                in1 = wi[:, 1].unsqueeze(1).broadcast_to([128, 2, K, OS])
                nc.vector.tensor_mul(cat, in0, in1)
                s2 = work.tile([128, 2, 2, OS], bf16, tag=f"s2{t}")
                nc.vector.tensor_add(s2, cat[:, :, 0:2], cat[:, :, 2:4])
                s = work.tile([128, 2, OS], bf16, tag=f"s{t}")
                nc.vector.tensor_add(s, s2[:, :, 0], s2[:, :, 1])
                s_t.append(s)

            # ---- |x| on ACT; min(x,0): tile 0 on DVE (fills a prep bubble),
            # tile 1 on the otherwise-idle Pool engine
            xa, xm = [], []
            for t in range(NT):
                xat = work.tile([128, BS], bf16, tag=f"xa{t}")
                nc.scalar.activation(xat, x_sb[t], AF.Abs)
                xa.append(xat)
                xmt = work.tile([128, BS], bf16, tag=f"xm{t}")
                eng = nc.vector if t == 0 else nc.gpsimd
                eng.tensor_scalar_min(xmt, x_sb[t], 0.0)
                xm.append(xmt)

            # ---- matmuls: psum half = batch half; weight-tile-0 matmuls
            # for both halves first so tile-1 prep overlaps them.
            psum = []
            for h in range(2):
                ps = pp.tile([128, HB], f32, tag=f"ps{h}")
                psum.append(ps)
            for t in range(NT):
                for h in range(2):
                    sl = slice(h * HB, (h + 1) * HB)
                    nc.tensor.matmul(psum[h], lhsT=s_t[t][:, 1], rhs=xa[t][:, sl],
                                     start=(t == 0), stop=False)
                    nc.tensor.matmul(psum[h], lhsT=s_t[t][:, 0], rhs=xm[t][:, sl],
                                     start=False, stop=(t == NT - 1))

            # ---- epilogues (half 0 on ACT, half 1 on DVE, in parallel) into
            # one SBUF tile, then a single store (one HWDGE descriptor-gen).
            out_sb = work.tile([128, BS], bf16, tag="outsb")
            nc.scalar.activation(out_sb[:, 0:HB], psum[0], AF.Identity,
                                 bias=b2_sb, scale=float(SCALE))
            nc.vector.scalar_tensor_tensor(
                out_sb[:, HB:BS], psum[1], float(SCALE),
                b2_sb.broadcast_to([OS, HB]), op0=OP.mult, op1=OP.add)
            nc.sync.dma_start(out=outt[:, :], in_=out_sb)

    nc.compile()
    _CACHE["nc"] = nc
    return nc


def _kan_numpy(x, w1, b1, w2, b2):
    # exact fallback, chunked over batch to bound memory
    out = np.empty((x.shape[0], w1.shape[0]), dtype=np.float32)
    d = (w1.shape[0] + w1.shape[1]) / 2
    s = 1.0 / np.sqrt(d * w1.shape[2])
    for lo in range(0, x.shape[0], 128):
        hi = min(lo + 128, x.shape[0])
        h = x[lo:hi, None, :, None] * w1[None] + b1[None]
        np.maximum(h, 0.0, out=h)
        out[lo:hi] = np.einsum("boik,oik->bo", h, w2) * s
    return out + b2[None, :]


def kernel(x, w1, b1, w2, b2):
    x = np.asarray(x, dtype=np.float32)
    w1 = np.asarray(w1, dtype=np.float32)
    b1 = np.asarray(b1, dtype=np.float32)
    w2 = np.asarray(w2, dtype=np.float32)
    b2 = np.asarray(b2, dtype=np.float32)

    if x.shape != (B, DIN) or w1.shape != (DOUT, DIN, K) or np.any(b1):
        return _kan_numpy(x, w1, b1, w2, b2)

    import ml_dtypes
    from concourse.bass_utils import run_bass_kernel_spmd

    nc = _build_nc()
    bf16 = ml_dtypes.bfloat16

    xT = np.ascontiguousarray(x.T).astype(bf16)          # (DIN, B)
    w1T = w1.transpose(1, 2, 0).astype(bf16)             # (DIN, K, DOUT)
    w2T = w2.transpose(1, 2, 0).astype(bf16)

    in_maps = []
    for core in range(N_CORES):
        bi, oj = divmod(core, OG)
        osl = slice(oj * OS, (oj + 1) * OS)
        wtc = np.empty((128, NT, 2, K, OS), dtype=bf16)
        for t in range(NT):
            isl = slice(t * 128, (t + 1) * 128)
            wtc[:, t, 0] = w1T[isl, :, osl]
            wtc[:, t, 1] = w2T[isl, :, osl]
        xtc = np.empty((128, NT, BS), dtype=bf16)
        for t in range(NT):
            xtc[:, t] = xT[t * 128:(t + 1) * 128, bi * BS:(bi + 1) * BS]
        in_maps.append({
            "wt": wtc,
            "xt": xtc,
            "b2s": np.ascontiguousarray(b2[osl], dtype=np.float32).reshape(OS, 1),
        })

    res = run_bass_kernel_spmd(nc, in_maps, core_ids=list(range(N_CORES)))

    out = np.empty((B, DOUT), dtype=np.float32)
    for core in range(N_CORES):
        bi, oj = divmod(core, OG)
        out[bi * BS:(bi + 1) * BS, oj * OS:(oj + 1) * OS] = \
            res.results[core]["outt"].astype(np.float32).T
    return out


# revision 27
# speedup vs baseline: 1.0316x; 1.0316x over previous
"""KAN layer kernel for TRN2, 8-core SPMD.

Math: out[b,o] = sum_{i,k} relu(x[b,i]*w1[o,i,k] + b1[o,i,k]) * w2[o,i,k] / 32 + b2[o]
With b1 == 0 (guaranteed by the generator) the relu factorizes:
    relu(x*w) = max(x,0)*max(w,0) + min(x,0)*min(w,0)
and with relu(-x) = relu(x) - x the layer collapses to two matmuls:
    T[o,i]  = sum_k w1*w2          S1[o,i] = sum_k relu(w1)*w2
    H = T - S1 ; G = S1 - H
    out = (relu(x) @ G^T + x @ H^T) / 32 + b2

Sharding: 4 batch groups x 2 dout groups (core = bi*2 + oj).
All tensors staged host-side in bf16 (halves DMA bytes, 1 cycle/row
matmuls, 2-4x DVE); accumulation in fp32 PSUM; bf16 output upcast on host.
relu(w1) runs at 4x DVE (tensor_scalar_max); one broadcast tensor_mul
computes w1*w2 and relu(w1)*w2 together at 2x.
"""

import numpy as np

B, DIN, DOUT, K = 2048, 256, 256, 4
N_CORES = 8
BG, OG = 4, 2                      # batch groups x dout groups
BS, OS = B // BG, DOUT // OG       # 512 batch rows, 128 dout cols per core
NT = DIN // 128                    # din tiles
HB = BS // 2                       # psum half of the batch shard
SCALE = 1.0 / np.sqrt(((DOUT + DIN) / 2) * K)   # 1/32
N_WARM = 40                        # PE warm-up matmuls ([128,128] each)

_CACHE = {}


def _build_nc():
    if "nc" in _CACHE:
        return _CACHE["nc"]
    import concourse.bacc as bacc
    import concourse.tile as tile
    from concourse import mybir

    f32 = mybir.dt.float32
    bf16 = mybir.dt.bfloat16
    AF = mybir.ActivationFunctionType
    OP = mybir.AluOpType

    nc = bacc.Bacc("TRN2", target_bir_lowering=False, debug=False,
                   num_devices=N_CORES, num_swdge_queues=2)
    # [i-part, i-tile, {w1,w2}, k, o]
    wt = nc.dram_tensor("wt", [128, NT, 2, K, OS], bf16, kind="ExternalInput")
    # [i-part, i-tile, b]
    xt = nc.dram_tensor("xt", [128, NT, BS], bf16, kind="ExternalInput")
    b2s = nc.dram_tensor("b2s", [OS, 1], f32, kind="ExternalInput")
    outt = nc.dram_tensor("outt", [OS, BS], bf16, kind="ExternalOutput")

    with tile.TileContext(nc) as tc:
        with (
            tc.tile_pool(name="io", bufs=1) as io,
            tc.tile_pool(name="work", bufs=1) as work,
            tc.tile_pool(name="pp", bufs=1, space="PSUM") as pp,
        ):
            # ---- DMA in: w0 first (feeds the serial DVE prep), x0 second
            # (feeds |x|/min(x,0) which fill DVE/ACT bubbles), then w1, x1.
            w_sb, x_sb = [], []
            for t in range(NT):
                # slots: 0=w1, 1=w2, 2=relu(w1) (filled on device)
                wi = io.tile([128, 3, K, OS], bf16, tag=f"w{t}")
                w_sb.append(wi)
                xi = io.tile([128, BS], bf16, tag=f"x{t}")
                x_sb.append(xi)
            # w0 split into w1/w2 slabs: relu(w1) starts one transfer earlier,
            # the pair-mult two; all weight copies precede x so no weight
            # consumer gets gated on an x transfer.
            nc.sync.dma_start(out=w_sb[0][:, 0:1], in_=wt[:, 0, 0:1])
            nc.sync.dma_start(out=w_sb[0][:, 1:2], in_=wt[:, 0, 1:2])
            nc.sync.dma_start(out=w_sb[1][:, 0:2], in_=wt[:, 1])
            nc.sync.dma_start(out=x_sb[0], in_=xt[:, 0])
            nc.sync.dma_start(out=x_sb[1], in_=xt[:, 1])
            b2_sb = io.tile([OS, 1], f32)
            nc.sync.dma_start(out=b2_sb, in_=b2s[:, :])

            # ---- PE warm-up: keep the tensor engine busy from t~0 so the
            # real matmuls run at full clock (p-state ramp needs ~3us busy).
            zt = work.tile([128, 128], bf16, tag="zt")
            nc.gpsimd.memset(zt, 0.0)
            pz = pp.tile([128, 128], f32, tag="pz")
            for i in range(N_WARM):
                nc.tensor.matmul(pz, lhsT=zt, rhs=zt, start=True, stop=True)

            # ---- weight prep (DVE):
            #   w1p = relu(w1)                       (tensor_scalar_max, 4x)
            #   cat[:,0]=w1*w2, cat[:,1]=w1p*w2      (one broadcast mul, 2x)
            #   k-sum -> s[:,0]=T=sum_k w1*w2, s[:,1]=S1=sum_k relu(w1)*w2
            # out = |x| @ S1^T + min(x,0) @ T^T  (no G/H subtracts needed)
            s_t = []
            for t in range(NT):
                wi = w_sb[t]
                nc.vector.tensor_scalar_max(wi[:, 2], wi[:, 0], 0.0)
                cat = work.tile([128, 2, K, OS], bf16, tag=f"cat{t}")
                in0 = wi[:, 0::2]
                in1 = wi[:, 1].unsqueeze(1).broadcast_to([128, 2, K, OS])
                nc.vector.tensor_mul(cat, in0, in1)
                s2 = work.tile([128, 2, 2, OS], bf16, tag=f"s2{t}")
                nc.vector.tensor_add(s2, cat[:, :, 0:2], cat[:, :, 2:4])
                s = work.tile([128, 2, OS], bf16, tag=f"s{t}")
                nc.vector.tensor_add(s, s2[:, :, 0], s2[:, :, 1])
                s_t.append(s)

            # ---- |x| on ACT; min(x,0): tile 0 on DVE (fills a prep bubble),
            # tile 1 on the otherwise-idle Pool engine
            xa, xm = [], []
            for t in range(NT):
                xat = work.tile([128, BS], bf16, tag=f"xa{t}")
                nc.scalar.activation(xat, x_sb[t], AF.Abs)
                xa.append(xat)
                xmt = work.tile([128, BS], bf16, tag=f"xm{t}")
                eng = nc.vector if t == 0 else nc.gpsimd
                eng.tensor_scalar_min(xmt, x_sb[t], 0.0)
                xm.append(xmt)

            # ---- matmuls: psum half = batch half; weight-tile-0 matmuls
            # for both halves first so tile-1 prep overlaps them.
            psum = []
            for h in range(2):
                ps = pp.tile([128, HB], f32, tag=f"ps{h}")
                psum.append(ps)
            for t in range(NT):
                for h in range(2):
                    sl = slice(h * HB, (h + 1) * HB)
                    nc.tensor.matmul(psum[h], lhsT=s_t[t][:, 1], rhs=xa[t][:, sl],
                                     start=(t == 0), stop=False)
                    nc.tensor.matmul(psum[h], lhsT=s_t[t][:, 0], rhs=xm[t][:, sl],
                                     start=False, stop=(t == NT - 1))

            # ---- epilogues (half 0 on ACT, half 1 on DVE, in parallel) into
            # one SBUF tile, then a single store (one HWDGE descriptor-gen).
            out_sb = work.tile([128, BS], bf16, tag="outsb")
            nc.scalar.activation(out_sb[:, 0:HB], psum[0], AF.Identity,
                                 bias=b2_sb, scale=float(SCALE))
            nc.vector.scalar_tensor_tensor(
                out_sb[:, HB:BS], psum[1], float(SCALE),
                b2_sb.broadcast_to([OS, HB]), op0=OP.mult, op1=OP.add)
            nc.sync.dma_start(out=outt[:, :], in_=out_sb)

    nc.compile()
    _CACHE["nc"] = nc
    return nc


def _kan_numpy(x, w1, b1, w2, b2):
    # exact fallback, chunked over batch to bound memory
    out = np.empty((x.shape[0], w1.shape[0]), dtype=np.float32)
    d = (w1.shape[0] + w1.shape[1]) / 2
    s = 1.0 / np.sqrt(d * w1.shape[2])
    for lo in range(0, x.shape[0], 128):
        hi = min(lo + 128, x.shape[0])
        h = x[lo:hi, None, :, None] * w1[None] + b1[None]
        np.maximum(h, 0.0, out=h)
        out[lo:hi] = np.einsum("boik,oik->bo", h, w2) * s
    return out + b2[None, :]


def kernel(x, w1, b1, w2, b2):
    x = np.asarray(x, dtype=np.float32)
    w1 = np.asarray(w1, dtype=np.float32)
    b1 = np.asarray(b1, dtype=np.float32)
    w2 = np.asarray(w2, dtype=np.float32)
    b2 = np.asarray(b2, dtype=np.float32)

    if x.shape != (B, DIN) or w1.shape != (DOUT, DIN, K) or np.any(b1):
        return _kan_numpy(x, w1, b1, w2, b2)

    import ml_dtypes
    from concourse.bass_utils import run_bass_kernel_spmd

    nc = _build_nc()
    bf16 = ml_dtypes.bfloat16

    xT = np.ascontiguousarray(x.T).astype(bf16)          # (DIN, B)
    w1T = w1.transpose(1, 2, 0).astype(bf16)             # (DIN, K, DOUT)
    w2T = w2.transpose(1, 2, 0).astype(bf16)

    in_maps = []
    for core in range(N_CORES):
        bi, oj = divmod(core, OG)
        osl = slice(oj * OS, (oj + 1) * OS)
        wtc = np.empty((128, NT, 2, K, OS), dtype=bf16)
        for t in range(NT):
            isl = slice(t * 128, (t + 1) * 128)
            wtc[:, t, 0] = w1T[isl, :, osl]
            wtc[:, t, 1] = w2T[isl, :, osl]
        xtc = np.empty((128, NT, BS), dtype=bf16)
        for t in range(NT):
            xtc[:, t] = xT[t * 128:(t + 1) * 128, bi * BS:(bi + 1) * BS]
        in_maps.append({
            "wt": wtc,
            "xt": xtc,
            "b2s": np.ascontiguousarray(b2[osl], dtype=np.float32).reshape(OS, 1),
        })

    res = run_bass_kernel_spmd(nc, in_maps, core_ids=list(range(N_CORES)))

    out = np.empty((B, DOUT), dtype=np.float32)
    for core in range(N_CORES):
        bi, oj = divmod(core, OG)
        out[bi * BS:(bi + 1) * BS, oj * OS:(oj + 1) * OS] = \
            res.results[core]["outt"].astype(np.float32).T
    return out


# revision 29
# speedup vs baseline: 1.0500x; 1.0178x over previous
"""KAN layer kernel for TRN2, 8-core SPMD.

Math: out[b,o] = sum_{i,k} relu(x[b,i]*w1[o,i,k] + b1[o,i,k]) * w2[o,i,k] / 32 + b2[o]
With b1 == 0 (guaranteed by the generator) the relu factorizes:
    relu(x*w) = max(x,0)*max(w,0) + min(x,0)*min(w,0)
and with relu(-x) = relu(x) - x the layer collapses to two matmuls:
    T[o,i]  = sum_k w1*w2          S1[o,i] = sum_k relu(w1)*w2
    H = T - S1 ; G = S1 - H
    out = (relu(x) @ G^T + x @ H^T) / 32 + b2

Sharding: 4 batch groups x 2 dout groups (core = bi*2 + oj).
All tensors staged host-side in bf16 (halves DMA bytes, 1 cycle/row
matmuls, 2-4x DVE); accumulation in fp32 PSUM; bf16 output upcast on host.
relu(w1) runs at 4x DVE (tensor_scalar_max); one broadcast tensor_mul
computes w1*w2 and relu(w1)*w2 together at 2x.
"""

import numpy as np

B, DIN, DOUT, K = 2048, 256, 256, 4
N_CORES = 8
BG, OG = 4, 2                      # batch groups x dout groups
BS, OS = B // BG, DOUT // OG       # 512 batch rows, 128 dout cols per core
NT = DIN // 128                    # din tiles
HB = BS // 2                       # psum half of the batch shard
SCALE = 1.0 / np.sqrt(((DOUT + DIN) / 2) * K)   # 1/32
N_WARM = 40                        # PE warm-up matmuls ([128,128] each)

_CACHE = {}


def _build_nc():
    if "nc" in _CACHE:
        return _CACHE["nc"]
    import concourse.bacc as bacc
    import concourse.tile as tile
    from concourse import mybir

    f32 = mybir.dt.float32
    bf16 = mybir.dt.bfloat16
    AF = mybir.ActivationFunctionType
    OP = mybir.AluOpType

    nc = bacc.Bacc("TRN2", target_bir_lowering=False, debug=False,
                   num_devices=N_CORES, num_swdge_queues=2)
    # [i-part, i-tile, {w1,w2}, k, o]
    wt = nc.dram_tensor("wt", [128, NT, 2, K, OS], bf16, kind="ExternalInput")
    # [i-part, i-tile, b]
    xt = nc.dram_tensor("xt", [128, NT, BS], bf16, kind="ExternalInput")
    b2s = nc.dram_tensor("b2s", [OS, 1], f32, kind="ExternalInput")
    outt = nc.dram_tensor("outt", [OS, BS], bf16, kind="ExternalOutput")

    with tile.TileContext(nc) as tc:
        with (
            tc.tile_pool(name="io", bufs=1) as io,
            tc.tile_pool(name="work", bufs=1) as work,
            tc.tile_pool(name="pp", bufs=1, space="PSUM") as pp,
        ):
            # ---- DMA in: w0 first (feeds the serial DVE prep), x0 second
            # (feeds |x|/min(x,0) which fill DVE/ACT bubbles), then w1, x1.
            w_sb, x_sb = [], []
            for t in range(NT):
                # slots: 0=w1, 1=w2, 2=relu(w1) (filled on device)
                wi = io.tile([128, 3, K, OS], bf16, tag=f"w{t}")
                w_sb.append(wi)
                xi = io.tile([128, BS], bf16, tag=f"x{t}")
                x_sb.append(xi)
            nc.sync.dma_start(out=w_sb[0][:, 0:2], in_=wt[:, 0])
            nc.sync.dma_start(out=x_sb[0], in_=xt[:, 0])
            nc.sync.dma_start(out=w_sb[1][:, 0:2], in_=wt[:, 1])
            nc.sync.dma_start(out=x_sb[1], in_=xt[:, 1])
            b2_sb = io.tile([OS, 1], f32)
            nc.sync.dma_start(out=b2_sb, in_=b2s[:, :])

            # ---- PE warm-up: keep the tensor engine busy from t~0 so the
            # real matmuls run at full clock (p-state ramp needs ~3us busy).
            zt = work.tile([128, 128], bf16, tag="zt")
            nc.gpsimd.memset(zt, 0.0)
            pz = pp.tile([128, 128], f32, tag="pz")
            for i in range(N_WARM):
                nc.tensor.matmul(pz, lhsT=zt, rhs=zt, start=True, stop=True)

            # ---- weight prep (DVE):
            #   w1p = relu(w1)                       (tensor_scalar_max, 4x)
            #   cat[:,0]=w1*w2, cat[:,1]=w1p*w2      (one broadcast mul, 2x)
            #   k-sum -> s[:,0]=T=sum_k w1*w2, s[:,1]=S1=sum_k relu(w1)*w2
            # out = |x| @ S1^T + min(x,0) @ T^T  (no G/H subtracts needed)
            s_t = []
            for t in range(NT):
                wi = w_sb[t]
                nc.vector.tensor_scalar_max(wi[:, 2], wi[:, 0], 0.0)
                cat = work.tile([128, 2, K, OS], bf16, tag=f"cat{t}")
                in0 = wi[:, 0::2]
                in1 = wi[:, 1].unsqueeze(1).broadcast_to([128, 2, K, OS])
                nc.vector.tensor_mul(cat, in0, in1)
                s2 = work.tile([128, 2, 2, OS], bf16, tag=f"s2{t}")
                nc.vector.tensor_add(s2, cat[:, :, 0:2], cat[:, :, 2:4])
                s = work.tile([128, 2, OS], bf16, tag=f"s{t}")
                nc.vector.tensor_add(s, s2[:, :, 0], s2[:, :, 1])
                s_t.append(s)

            # ---- |x| on ACT; min(x,0) on the otherwise-idle Pool engine
            # (keeps the whole DVE serial budget for the weight prep)
            xa, xm = [], []
            for t in range(NT):
                xat = work.tile([128, BS], bf16, tag=f"xa{t}")
                nc.scalar.activation(xat, x_sb[t], AF.Abs)
                xa.append(xat)
                xmt = work.tile([128, BS], bf16, tag=f"xm{t}")
                nc.gpsimd.tensor_scalar_min(xmt, x_sb[t], 0.0)
                xm.append(xmt)

            # ---- matmuls: psum half = batch half; weight-tile-0 matmuls
            # for both halves first so tile-1 prep overlaps them.
            psum = []
            for h in range(2):
                ps = pp.tile([128, HB], f32, tag=f"ps{h}")
                psum.append(ps)
            for t in range(NT):
                for h in range(2):
                    sl = slice(h * HB, (h + 1) * HB)
                    nc.tensor.matmul(psum[h], lhsT=s_t[t][:, 1], rhs=xa[t][:, sl],
                                     start=(t == 0), stop=False)
                    nc.tensor.matmul(psum[h], lhsT=s_t[t][:, 0], rhs=xm[t][:, sl],
                                     start=False, stop=(t == NT - 1))

            # ---- epilogues (half 0 on ACT, half 1 on DVE, in parallel) into
            # one SBUF tile, then a single store (one HWDGE descriptor-gen).
            out_sb = work.tile([128, BS], bf16, tag="outsb")
            nc.scalar.activation(out_sb[:, 0:HB], psum[0], AF.Identity,
                                 bias=b2_sb, scale=float(SCALE))
            nc.vector.scalar_tensor_tensor(
                out_sb[:, HB:BS], psum[1], float(SCALE),
                b2_sb.broadcast_to([OS, HB]), op0=OP.mult, op1=OP.add)
            nc.sync.dma_start(out=outt[:, :], in_=out_sb)

    nc.compile()
    _CACHE["nc"] = nc
    return nc


def _kan_numpy(x, w1, b1, w2, b2):
    # exact fallback, chunked over batch to bound memory
    out = np.empty((x.shape[0], w1.shape[0]), dtype=np.float32)
    d = (w1.shape[0] + w1.shape[1]) / 2
    s = 1.0 / np.sqrt(d * w1.shape[2])
    for lo in range(0, x.shape[0], 128):
        hi = min(lo + 128, x.shape[0])
        h = x[lo:hi, None, :, None] * w1[None] + b1[None]
        np.maximum(h, 0.0, out=h)
        out[lo:hi] = np.einsum("boik,oik->bo", h, w2) * s
    return out + b2[None, :]


def kernel(x, w1, b1, w2, b2):
    x = np.asarray(x, dtype=np.float32)
    w1 = np.asarray(w1, dtype=np.float32)
    b1 = np.asarray(b1, dtype=np.float32)
    w2 = np.asarray(w2, dtype=np.float32)
    b2 = np.asarray(b2, dtype=np.float32)

    if x.shape != (B, DIN) or w1.shape != (DOUT, DIN, K) or np.any(b1):
        return _kan_numpy(x, w1, b1, w2, b2)

    import ml_dtypes
    from concourse.bass_utils import run_bass_kernel_spmd

    nc = _build_nc()
    bf16 = ml_dtypes.bfloat16

    xT = np.ascontiguousarray(x.T).astype(bf16)          # (DIN, B)
    w1T = w1.transpose(1, 2, 0).astype(bf16)             # (DIN, K, DOUT)
    w2T = w2.transpose(1, 2, 0).astype(bf16)

    in_maps = []
    for core in range(N_CORES):
        bi, oj = divmod(core, OG)
        osl = slice(oj * OS, (oj + 1) * OS)
        wtc = np.empty((128, NT, 2, K, OS), dtype=bf16)
        for t in range(NT):
            isl = slice(t * 128, (t + 1) * 128)
            wtc[:, t, 0] = w1T[isl, :, osl]
            wtc[:, t, 1] = w2T[isl, :, osl]
        xtc = np.empty((128, NT, BS), dtype=bf16)
        for t in range(NT):
            xtc[:, t] = xT[t * 128:(t + 1) * 128, bi * BS:(bi + 1) * BS]
        in_maps.append({
            "wt": wtc,
            "xt": xtc,
            "b2s": np.ascontiguousarray(b2[osl], dtype=np.float32).reshape(OS, 1),
        })

    res = run_bass_kernel_spmd(nc, in_maps, core_ids=list(range(N_CORES)))

    out = np.empty((B, DOUT), dtype=np.float32)
    for core in range(N_CORES):
        bi, oj = divmod(core, OG)
        out[bi * BS:(bi + 1) * BS, oj * OS:(oj + 1) * OS] = \
            res.results[core]["outt"].astype(np.float32).T
    return out
